# revision 18
# baseline (speedup 1.0000x reference)
"""Multi-head attention (RoPE, causal) Trainium2 kernel, SPMD over 8 NeuronCores.

Problem: x[2,2048,1024] @ {W_q,W_k,W_v}[1024,1024] -> 16-head causal attention
with RoPE -> @ W_o[1024,1024].

Sharding (batch x heads): core c handles batch b=c//4 and head group g=c%4
(4 heads = 256 of the 1024 qkv dims). Each core computes its heads' QKV
projections, RoPE, causal attention, and a partial out-projection
(ctx_g @ W_o[256g:256g+256, :]). The host sums the 4 partials per batch
(unshard of a partial-sum sharding) and transposes back.

On-device layout is fully transposed ([feature, seq]) so no transposes are
needed anywhere: scores are computed as scoresT[k,q] = K^T.T @ Q^T, the
softmax denominator falls out of the AV matmul via a ones-column appended to
V, and the out-projection consumes ctxT directly.

v2 schedule, all bf16 (fp8 was measured to blow the 2e-2 error budget):
- Causal diagonal restriction: for the 4 diagonal key-blocks of each query
  block, scores/exp/AV only cover the valid q >= k column range (the first
  128 columns of the range get a [128,128] triangle mask); saves PE, ACT and
  DVE work and shrinks the mask input to 64KB.
- Fine-grained software pipeline: attention k-block steps are interleaved
  with "filler" quanta (QK projection halves, V projections, out-projection
  chunks) so the in-order PE queue always has independent work while the
  ACT engine exponentiates. The boot phase runs 8 parallel PSUM accumulation
  chains (q/k for both head pairs + 4 V blocks) paced by the x column-block
  DMA arrivals, with clock-warm matmuls on a memset tile before that.
- Softmax denominators: engine copies stage the PSUM ones-row to SBUF, one
  DMA repartitions both heads to [128,8] (reciprocal cost scales with free
  size only -> ~free), one DMA bounces to DRAM, one stride-0 DMA broadcasts
  across partitions.
"""

import numpy as np
import ml_dtypes

B = 2
S = 2048
D = 1024
H = 16
HD = 64
N_CORES = 8
H_PER_CORE = 4
DQ = H_PER_CORE * HD  # 256 qkv dims per core
N_DC = D // 128  # 8 contraction chunks
N_SB = S // 512  # 4 seq blocks of 512
N_KB = S // 128  # 16 key blocks of 128
THETA = 10000.0

_CACHED = None


def _build_kernel():
    import concourse.bass as bass
    import concourse.mybir as mybir
    import concourse.tile as tile
    from concourse import bacc

    f32 = mybir.dt.float32
    bf16 = mybir.dt.bfloat16

    nc = bacc.Bacc(None, target_bir_lowering=False, num_devices=N_CORES)

    xT = nc.dram_tensor("xT", [D, S], bf16, kind="ExternalInput")
    wq = nc.dram_tensor("wq", [D, DQ], bf16, kind="ExternalInput")
    wk = nc.dram_tensor("wk", [D, DQ], bf16, kind="ExternalInput")
    wv = nc.dram_tensor("wv", [D, DQ], bf16, kind="ExternalInput")
    wo = nc.dram_tensor("wo", [DQ, D], bf16, kind="ExternalInput")
    cosT = nc.dram_tensor("cosT", [128, S], f32, kind="ExternalInput")
    sinT = nc.dram_tensor("sinT", [128, S], f32, kind="ExternalInput")
    # masks[k, 128*c + q] = 1.0 if k <= q else 0 (c=0,1 identical copies)
    masks = nc.dram_tensor("masks", [128, 256], bf16, kind="ExternalInput")
    yT = nc.dram_tensor("yT", [D, S], bf16, kind="ExternalOutput")

    with tile.TileContext(nc) as tc:
        with (
            tc.tile_pool(name="persist", bufs=1) as persist,
            tc.tile_pool(name="attn", bufs=8) as attn_pool,
            tc.tile_pool(name="rope", bufs=4) as rope_pool,
            tc.tile_pool(name="small", bufs=4) as small_pool,
            tc.tile_pool(name="yout", bufs=3) as yout_pool,
            tc.tile_pool(name="dram", bufs=1, space="DRAM") as dram_pool,
            tc.tile_pool(name="psA", bufs=2, space="PSUM") as psA,  # scores 2-bank
            tc.tile_pool(name="psB", bufs=2, space="PSUM") as psB,  # ctx accum
            tc.tile_pool(name="psC", bufs=2, space="PSUM") as psC,  # proj/y
        ):
            # ---------------- input DMA ----------------
            # weights first (boot chains + warm-up), then x seq-block 0 in
            # per-dc chunks (fine-grained boot pacing), then rope tables for
            # block 0, then the x remainder per dc, then the rest.
            wq_sb = persist.tile([128, N_DC, DQ], bf16, tag="wq")
            nc.sync.dma_start(
                out=wq_sb[:], in_=wq.rearrange("(c p) n -> p c n", p=128)
            )
            wk_sb = persist.tile([128, N_DC, DQ], bf16, tag="wk")
            nc.gpsimd.dma_start(
                out=wk_sb[:], in_=wk.rearrange("(c p) n -> p c n", p=128)
            )
            wv_sb = persist.tile([128, N_DC, DQ], bf16, tag="wv")
            nc.sync.dma_start(
                out=wv_sb[:], in_=wv.rearrange("(c p) n -> p c n", p=128)
            )
            xt_sb = [
                persist.tile([128, S], bf16, tag=f"xt{dc}", name=f"xt{dc}")
                for dc in range(N_DC)
            ]
            for dc in range(N_DC):
                eng = nc.sync if dc % 2 == 0 else nc.gpsimd
                eng.dma_start(
                    out=xt_sb[dc][:, 0:512],
                    in_=xT[128 * dc : 128 * (dc + 1), 0:512],
                )
            cos_sb = persist.tile([128, S], f32, tag="cos")
            sin_sb = persist.tile([128, S], f32, tag="sin")
            nc.sync.dma_start(out=cos_sb[:, 0:512], in_=cosT[:, 0:512])
            nc.gpsimd.dma_start(out=sin_sb[:, 0:512], in_=sinT[:, 0:512])
            mask_sb = persist.tile([128, 256], bf16, tag="mask")
            nc.gpsimd.dma_start(out=mask_sb[:], in_=masks[:])
            for dc in range(N_DC):
                eng = nc.sync if dc % 2 == 0 else nc.gpsimd
                eng.dma_start(
                    out=xt_sb[dc][:, 512:S],
                    in_=xT[128 * dc : 128 * (dc + 1), 512:S],
                )
            nc.sync.dma_start(out=cos_sb[:, 512:S], in_=cosT[:, 512:S])
            nc.gpsimd.dma_start(out=sin_sb[:, 512:S], in_=sinT[:, 512:S])
            wo_sb = persist.tile([128, 2, D], bf16, tag="wo")
            nc.sync.dma_start(
                out=wo_sb[:], in_=wo.rearrange("(c p) n -> p c n", p=128)
            )

            # PE warm-up on a memset tile: no DMA dependency, so the HAM
            # clock ramp (needs ~3.4us of sustained activity for 2.4GHz)
            # starts as soon as the preamble ends, while inputs stream.
            junk = persist.tile([128, 256], bf16, tag="junk")
            nc.vector.memset(junk[:], 0.03125)
            warm_tiles = [
                psA.tile([128, 1024], f32, tag="score", name=f"warm{i}")
                for i in range(2)
            ]
            for wi in range(24):
                nc.tensor.matmul(
                    warm_tiles[wi % 2][:, 0:256],
                    junk[:, 0:128],
                    junk[:],
                    start=True,
                    stop=True,
                )

            # persistent intermediates
            qT_sb = persist.tile([128, 2, S], bf16, tag="qT")  # [64h..., cc, s]
            kT_sb = persist.tile([128, 2, S], bf16, tag="kT")
            v_sb = persist.tile([128, N_KB, H_PER_CORE, HD + 1], bf16, tag="v")
            nc.vector.memset(v_sb[:, :, :, HD : HD + 1], 1.0)
            ctxT_sb = persist.tile([128, 2, S], bf16, tag="ctxT")  # unnormalized
            # denominators staged on one partition (engine writes must start
            # at partition 0/32/64/96), pre-interleaved as [p, 4h+j] so the
            # repartition DMA to [128,8] is a plain row-major linearize
            stage_sb = persist.tile([1, 2 * N_SB, 128, 8], f32, tag="stage")
            recip_dram = dram_pool.tile([N_SB, 2, 1024], bf16, tag="rdram")

            # ---------------- helpers ----------------
            def rope(src_ps, dst_sb, cc, sb):
                """dst = src*cos + rotate_half(src)*sin, fp32 in, bf16 out.

                The rotate-half partition shift is done by small SBUF->SBUF
                DMAs (a [32,512] DVE op costs as much as a [128,512] one, so
                quarter-sized DVE ops waste 3/4 of the lanes; DMA engines are
                otherwise idle).
                """
                ss = slice(512 * sb, 512 * (sb + 1))
                t1 = rope_pool.tile([128, 512], bf16, tag="ropeA", name="t1")
                nc.vector.tensor_mul(t1[:], src_ps, cos_sb[:, ss])
                # sin table is pre-shifted on the host (sinx[p] =
                # sin_signed[partner(p)]) so this product is computed at the
                # SOURCE rows and only then moved to the partner rows by DMA
                t2p = rope_pool.tile([128, 512], bf16, tag="ropeQ", name="t2p")
                nc.vector.tensor_mul(t2p[:], src_ps, sin_sb[:, ss])
                rot = rope_pool.tile([128, 512], bf16, tag="ropeB", name="rot")
                for quarter in range(4):
                    o = 32 * quarter
                    src_o = o + 32 if quarter % 2 == 0 else o - 32
                    nc.gpsimd.dma_start(
                        out=rot[o : o + 32, :], in_=t2p[src_o : src_o + 32, :]
                    )
                nc.vector.tensor_add(dst_sb[:, cc, ss], t1[:], rot[:])

            def proj_q_quantum(cc, sb):
                ss = slice(512 * sb, 512 * (sb + 1))
                q_ps = psC.tile([128, 512], f32, tag="proj", name="q_ps")
                for dc in range(N_DC):
                    nc.tensor.matmul(
                        q_ps[:],
                        wq_sb[:, dc, 128 * cc : 128 * (cc + 1)],
                        xt_sb[dc][:, ss],
                        start=(dc == 0),
                        stop=(dc == N_DC - 1),
                    )
                rope(q_ps[:], qT_sb, cc, sb)

            def proj_k_quantum(cc, sb):
                ss = slice(512 * sb, 512 * (sb + 1))
                k_ps = psC.tile([128, 512], f32, tag="proj", name="k_ps")
                for dc in range(N_DC):
                    nc.tensor.matmul(
                        k_ps[:],
                        wk_sb[:, dc, 128 * cc : 128 * (cc + 1)],
                        xt_sb[dc][:, ss],
                        start=(dc == 0),
                        stop=(dc == N_DC - 1),
                    )
                rope(k_ps[:], kT_sb, cc, sb)

            def proj_v_quantum(sc):
                # PSUM accumulation groups are per-bank, so each sc chain
                # needs its own tile even though it only fills half of it
                v_ps = psC.tile([128, 512], f32, tag="proj", name="v_ps")[
                    :, 0:256
                ]
                for dc in range(N_DC):
                    nc.tensor.matmul(
                        v_ps,
                        xt_sb[dc][:, 128 * sc : 128 * (sc + 1)],
                        wv_sb[:, dc, :],
                        start=(dc == 0),
                        stop=(dc == N_DC - 1),
                    )
                nc.vector.tensor_copy(
                    v_sb[:, sc, :, 0:HD],
                    v_ps.rearrange("p (h d) -> p h d", h=H_PER_CORE),
                )

            def out_proj_quantum(qb, oc, copy_eng=None):
                qs = slice(512 * qb, 512 * (qb + 1))
                y_ps = psC.tile([128, 512], f32, tag="proj", name="y_ps")
                for cc in range(2):
                    nc.tensor.matmul(
                        y_ps[:],
                        wo_sb[:, cc, 128 * oc : 128 * (oc + 1)],
                        ctxT_sb[:, cc, qs],
                        start=(cc == 0),
                        stop=(cc == 1),
                    )
                y_sb = yout_pool.tile([128, 512], bf16, tag="y", name="y_sb")
                nc.vector.tensor_copy(y_sb[:], y_ps[:])
                nc.sync.dma_start(
                    out=yT[128 * oc : 128 * (oc + 1), qs], in_=y_sb[:]
                )

            def attention(cc, qb, fills=()):
                """Causal attention for head pair cc, query block qb.

                Per k-block: two score matmuls (head h in PE row-group h) into
                one [128,1024] PSUM tile, one exp over both heads, triangle
                mask on the diagonal 128 columns, then (one k-block delayed)
                the two AV matmuls accumulating ctx+denominator via the ones
                column. Diagonal k-blocks only touch the valid q >= k range.

                `fills` are independent PE-work quanta, distributed across
                k-block slots to cover the exp dependency chain.
                """
                import collections

                fills = collections.deque(fills)
                nkb = 4 * qb + 4
                ctx_ps = [
                    psB.tile([HD + 1, 512], f32, tag="ctx", name=f"ctx{h}")
                    for h in range(2)
                ]
                pending = None  # (kb, o, a_t) whose AV matmuls haven't run

                def emit_av(kb, o, a_t, stop):
                    for h in range(2):
                        nc.tensor.matmul(
                            ctx_ps[h][:, o:512],
                            v_sb[:, kb, 2 * cc + h, :],
                            a_t[:, 512 * h + o : 512 * (h + 1)],
                            start=(kb == 0),
                            stop=stop,
                            skip_group_check=True,
                        )

                for kb in range(nkb):
                    j = kb - 4 * qb  # >= 0 on diagonal blocks
                    o = 128 * j if j > 0 else 0
                    s_ps = psA.tile([128, 1024], f32, tag="score", name="s_ps")
                    for h in range(2):
                        hp = slice(64 * h, 64 * (h + 1))
                        nc.tensor.matmul(
                            s_ps[:, 512 * h + o : 512 * (h + 1)],
                            kT_sb[hp, cc, 128 * kb : 128 * (kb + 1)],
                            qT_sb[hp, cc, 512 * qb + o : 512 * (qb + 1)],
                            start=True,
                            stop=True,
                        )
                    a_t = attn_pool.tile([128, 1024], bf16, tag="attnT", name="a_t")
                    s3 = s_ps[:].rearrange("p (h q) -> p h q", h=2)
                    a3 = a_t[:].rearrange("p (h q) -> p h q", h=2)
                    nc.scalar.activation(
                        a3[:, :, o:512],
                        s3[:, :, o:512],
                        mybir.ActivationFunctionType.Exp,
                        scale=float(1.0 / np.sqrt(HD)),
                    )
                    if j >= 0:
                        nc.vector.tensor_mul(
                            a3[:, :, o : o + 128],
                            a3[:, :, o : o + 128],
                            mask_sb[:].rearrange("p (c q) -> p c q", c=2),
                        )
                    if pending is not None:
                        emit_av(*pending, stop=False)
                    pending = (kb, o, a_t)
                    # distribute remaining fills over remaining slots
                    nf = (len(fills) + nkb - 1 - kb) // (nkb - kb)
                    for _ in range(nf):
                        fills.popleft()()
                emit_av(*pending, stop=True)
                # stage denominators first (the normalization chain hangs off
                # them), then remaining fills, then the bulk ctx copies
                call = qb * 2 + cc
                for h in range(2):
                    crow = ctx_ps[h][HD : HD + 1, :]
                    src = bass.AP(
                        tensor=crow.tensor,
                        offset=crow.offset,
                        ap=[list(crow.ap)[0], [1, 128], [128, 4]],
                    )
                    if h == 0:
                        nc.vector.tensor_copy(
                            stage_sb[0:1, call, :, 4 * h : 4 * h + 4], src
                        )
                    else:
                        nc.scalar.copy(
                            stage_sb[0:1, call, :, 4 * h : 4 * h + 4], src
                        )
                while fills:
                    fills.popleft()()
                for h in range(2):
                    nc.vector.tensor_copy(
                        ctxT_sb[64 * h : 64 * (h + 1), cc, 512 * qb : 512 * (qb + 1)],
                        ctx_ps[h][0:HD, :],
                    )

            def normalize(cc, qb):
                """Reciprocal + broadcast + scale for head pair cc, block qb.

                stage[1, 1024] -> den[128, 8] by DMA (den[p, 4h+j] =
                stage[512h + 128j + p]) so the reciprocal's free size is 8,
                then DMA to a DRAM bounce and one stride-0 partition-broadcast
                DMA back to [128,512] (64 rows per head).
                """
                call = qb * 2 + cc
                den = small_pool.tile([128, 8], f32, tag="den", name="den")
                nc.sync.dma_start(out=den[:], in_=stage_sb[0:1, call, :, :])
                rec = small_pool.tile([128, 8], bf16, tag="rec", name="rec")
                with nc.allow_low_precision(
                    reason="bf16 softmax denom matches bf16 attn weights"
                ):
                    nc.vector.reciprocal(rec[:], den[:])
                if cc == 1 and qb == N_SB - 1:
                    # keep the PE's HAM clock warm through the tail
                    # normalization chain: scratch matmuls gated on the
                    # chain's own data so the scheduler cannot hoist them
                    warm = psA.tile([128, 1024], f32, tag="score", name="warm")
                    for wi in range(8):
                        nc.tensor.matmul(
                            warm[0:8, 0:512],
                            rec[:],
                            xt_sb[wi][:, 0:512],
                            start=True,
                            stop=True,
                        )
                drow = recip_dram[qb, cc, :]
                dst = bass.AP(
                    tensor=drow.tensor,
                    offset=drow.offset,
                    ap=[[1, 128], [512, 2], [128, 4]],
                )
                nc.sync.dma_start(
                    out=dst, in_=rec[:].rearrange("p (h j) -> p h j", h=2)
                )
                qs = slice(512 * qb, 512 * (qb + 1))
                bc_sb = small_pool.tile([128, 512], bf16, tag="bcast", name="bc_sb")
                bsrc = bass.AP(
                    tensor=drow.tensor,
                    offset=drow.offset,
                    ap=[[512, 2], [0, 64], [1, 512]],
                )
                nc.sync.dma_start(out=bc_sb[:], in_=bsrc)
                nc.vector.tensor_mul(
                    ctxT_sb[:, cc, qs], ctxT_sb[:, cc, qs], bc_sb[:]
                )

            # ---------------- boot: projections paced by the x DMAs -------
            # 8 parallel PSUM chains (q/k for both head pairs in the two psA
            # score tiles, 4 V blocks in the two psC tiles) so each arriving
            # x chunk feeds ~1.3us of PE work.
            qk_ps = [
                psA.tile([128, 1024], f32, tag="score", name=f"qk{cc}")
                for cc in range(2)
            ]
            v_boot = [
                psC.tile([128, 512], f32, tag="proj", name=f"vb{i}")
                for i in range(2)
            ]
            for dc in range(N_DC):
                st, sp = dc == 0, dc == N_DC - 1
                for cc in range(2):
                    nc.tensor.matmul(
                        qk_ps[cc][:, 0:512],
                        wq_sb[:, dc, 128 * cc : 128 * (cc + 1)],
                        xt_sb[dc][:, 0:512],
                        start=st,
                        stop=sp,
                    )
                    nc.tensor.matmul(
                        qk_ps[cc][:, 512:1024],
                        wk_sb[:, dc, 128 * cc : 128 * (cc + 1)],
                        xt_sb[dc][:, 0:512],
                        start=st,
                        stop=sp,
                    )
                # only 2 spare banks -> first 2 of the 4 V chains ride along
                for sc in range(2):
                    nc.tensor.matmul(
                        v_boot[sc][:, 0:256],
                        xt_sb[dc][:, 128 * sc : 128 * (sc + 1)],
                        wv_sb[:, dc, :],
                        start=st,
                        stop=sp,
                    )
            for cc in range(2):
                rope(qk_ps[cc][:, 0:512], qT_sb, cc, 0)
                rope(qk_ps[cc][:, 512:1024], kT_sb, cc, 0)
            for sc in range(2):
                nc.vector.tensor_copy(
                    v_sb[:, sc, :, 0:HD],
                    v_boot[sc][:, 0:256].rearrange(
                        "p (h d) -> p h d", h=H_PER_CORE
                    ),
                )
            for sc in range(2, 4):
                proj_v_quantum(sc)

            # ---------------- main pipeline ----------------
            # baseline-style bulk emission order: the ACT-paced attention
            # inner loop leaves the PE ~15-25% idle, which keeps the HAM
            # power governor's activity estimate under its clamp threshold.
            # A densely filler-packed schedule measures ~40% SLOWER: the
            # governor duty-cycles the PE to half rate (k=4/n=8 windows,
            # matmuls stretch ~1.6x) for half the kernel.
            for sb in range(N_SB):
                if sb > 0:
                    proj_q_quantum(1, sb)
                    proj_k_quantum(1, sb)
                    for sc in range(4 * sb, 4 * sb + 4):
                        proj_v_quantum(sc)
                attention(0, sb)
                normalize(0, sb)  # chain covered by attention(1, sb) PE work
                if sb == N_SB - 1:
                    attention(
                        1,
                        sb,
                        fills=[
                            (lambda oc=oc: out_proj_quantum(2, oc))
                            for oc in range(4, N_DC)
                        ],
                    )
                else:
                    attention(1, sb)
                normalize(1, sb)
                if sb < N_SB - 1:
                    # emit the next block's first projection before this
                    # block's out-projection so the reciprocal chain is
                    # covered by PE work and the PE never idles
                    proj_q_quantum(0, sb + 1)
                    proj_k_quantum(0, sb + 1)
                if sb < 2:
                    for oc in range(N_DC):
                        out_proj_quantum(sb, oc)
                elif sb == 2:
                    # hold back half of qb=2's out-projection; it is emitted
                    # as filler inside attention(1, 3) to cover the final
                    # normalization chain
                    for oc in range(0, 4):
                        out_proj_quantum(2, oc)
                else:
                    for oc in range(N_DC):
                        out_proj_quantum(3, oc)

    nc.compile()
    return nc


def _rope_tables():
    inv_freq = (
        1.0 / (THETA ** (np.arange(0, HD, 2, dtype=np.float32) / HD))
    ).astype(np.float32)
    pos = np.arange(S, dtype=np.float32)
    ang = pos[:, None] * inv_freq[None, :]  # [S, 32]
    cos_half = np.cos(ang).astype(np.float32).T  # [32, S]
    sin_half = np.sin(ang).astype(np.float32).T
    # per-head 64 rows: cos rows duplicated. The sin table is PRE-SHIFTED:
    # row p holds sin_signed[partner(p)] (partner = rotate-half swap), so the
    # kernel multiplies at the source rows and a plain partition-shift DMA
    # finishes rotate-half: sinx per head = (+sin | -sin).
    cos64 = np.concatenate([cos_half, cos_half], axis=0)
    sinx64 = np.concatenate([sin_half, -sin_half], axis=0)
    cosT = np.concatenate([cos64, cos64], axis=0)  # [128, S] two heads
    sinT = np.concatenate([sinx64, sinx64], axis=0)
    return np.ascontiguousarray(cosT), np.ascontiguousarray(sinT)


def _masks():
    k = np.arange(128)[:, None]
    q = np.arange(128)[None, :]
    tri = (k <= q).astype(ml_dtypes.bfloat16)
    m = np.empty((128, 256), dtype=ml_dtypes.bfloat16)
    m[:, 0:128] = tri
    m[:, 128:256] = tri
    return m


def kernel(x, W_q, W_k, W_v, W_o):
    global _CACHED
    from concourse.bass_utils import run_bass_kernel_spmd

    if _CACHED is None:
        _CACHED = _build_kernel()
    nc = _CACHED

    bf = ml_dtypes.bfloat16
    cosT, sinT = _rope_tables()
    masks = _masks()
    x = np.asarray(x)
    W_q, W_k, W_v, W_o = (np.asarray(w) for w in (W_q, W_k, W_v, W_o))
    xT = [np.ascontiguousarray(x[b].T).astype(bf) for b in range(B)]

    in_maps = []
    for c in range(N_CORES):
        b, g = divmod(c, 4)
        cols = slice(DQ * g, DQ * (g + 1))
        in_maps.append(
            {
                "xT": xT[b],
                "wq": np.ascontiguousarray(W_q[:, cols]).astype(bf),
                "wk": np.ascontiguousarray(W_k[:, cols]).astype(bf),
                "wv": np.ascontiguousarray(W_v[:, cols]).astype(bf),
                "wo": np.ascontiguousarray(W_o[cols, :]).astype(bf),
                "cosT": cosT,
                "sinT": sinT,
                "masks": masks,
            }
        )

    res = run_bass_kernel_spmd(nc, in_maps, core_ids=list(range(N_CORES)))
    kernel.last_results = res

    y = np.empty((B, S, D), dtype=np.float32)
    for b in range(B):
        acc = res.results[4 * b]["yT"].astype(np.float32)
        for g in range(1, 4):
            acc += res.results[4 * b + g]["yT"].astype(np.float32)
        y[b] = acc.T
    return y


# revision 23
# speedup vs baseline: 1.6141x; 1.6141x over previous
"""Multi-head attention (RoPE, causal) Trainium2 kernel, SPMD over 8 NeuronCores.

Problem: x[2,2048,1024] @ {W_q,W_k,W_v}[1024,1024] -> 16-head causal attention
with RoPE -> @ W_o[1024,1024].

Sharding (batch x heads): core c handles batch b=c//4 and head group g=c%4
(4 heads = 256 of the 1024 qkv dims). Each core computes its heads' QKV
projections, RoPE, causal attention, and a partial out-projection
(ctx_g @ W_o[256g:256g+256, :]). The host sums the 4 partials per batch
(unshard of a partial-sum sharding) and transposes back.

On-device layout is fully transposed ([feature, seq]) so no transposes are
needed anywhere: scores are computed as scoresT[k,q] = K^T.T @ Q^T, the
softmax denominator falls out of the AV matmul via a ones-column appended to
V, and the out-projection consumes ctxT directly.

v2 schedule, all bf16 (fp8 was measured to blow the 2e-2 error budget):
- Causal diagonal restriction: for the 4 diagonal key-blocks of each query
  block, scores/exp/AV only cover the valid q >= k column range (the first
  128 columns of the range get a [128,128] triangle mask); saves PE, ACT and
  DVE work and shrinks the mask input to 64KB.
- Fine-grained software pipeline: attention k-block steps are interleaved
  with "filler" quanta (QK projection halves, V projections, out-projection
  chunks) so the in-order PE queue always has independent work while the
  ACT engine exponentiates. The boot phase runs 8 parallel PSUM accumulation
  chains (q/k for both head pairs + 4 V blocks) paced by the x column-block
  DMA arrivals, with clock-warm matmuls on a memset tile before that.
- Softmax denominators: engine copies stage the PSUM ones-row to SBUF, one
  DMA repartitions both heads to [128,8] (reciprocal cost scales with free
  size only -> ~free), one DMA bounces to DRAM, one stride-0 DMA broadcasts
  across partitions.
"""

import numpy as np
import ml_dtypes

B = 2
S = 2048
D = 1024
H = 16
HD = 64
N_CORES = 8
H_PER_CORE = 4
DQ = H_PER_CORE * HD  # 256 qkv dims per core
N_DC = D // 128  # 8 contraction chunks
N_SB = S // 512  # 4 seq blocks of 512
N_KB = S // 128  # 16 key blocks of 128
THETA = 10000.0

_CACHED = None


def _build_kernel():
    import concourse.bass as bass
    import concourse.mybir as mybir
    import concourse.tile as tile
    from concourse import bacc

    f32 = mybir.dt.float32
    bf16 = mybir.dt.bfloat16

    nc = bacc.Bacc(None, target_bir_lowering=False, num_devices=N_CORES)

    xT = nc.dram_tensor("xT", [D, S], bf16, kind="ExternalInput")
    wq = nc.dram_tensor("wq", [D, DQ], bf16, kind="ExternalInput")
    wk = nc.dram_tensor("wk", [D, DQ], bf16, kind="ExternalInput")
    wv = nc.dram_tensor("wv", [D, DQ], bf16, kind="ExternalInput")
    wo = nc.dram_tensor("wo", [DQ, D], bf16, kind="ExternalInput")
    cosT = nc.dram_tensor("cosT", [128, S], f32, kind="ExternalInput")
    sinT = nc.dram_tensor("sinT", [128, S], f32, kind="ExternalInput")
    # masks[k, 128*c + q] = 1.0 if k <= q else 0 (c=0,1 identical copies)
    masks = nc.dram_tensor("masks", [128, 256], bf16, kind="ExternalInput")
    yT = nc.dram_tensor("yT", [D, S], bf16, kind="ExternalOutput")

    with tile.TileContext(nc) as tc:
        with (
            tc.tile_pool(name="persist", bufs=1) as persist,
            tc.tile_pool(name="attn", bufs=8) as attn_pool,
            tc.tile_pool(name="rope", bufs=4) as rope_pool,
            tc.tile_pool(name="small", bufs=4) as small_pool,
            tc.tile_pool(name="yout", bufs=3) as yout_pool,
            tc.tile_pool(name="dram", bufs=1, space="DRAM") as dram_pool,
            tc.tile_pool(name="psA", bufs=2, space="PSUM") as psA,  # scores 2-bank
            tc.tile_pool(name="psB", bufs=2, space="PSUM") as psB,  # ctx accum
            tc.tile_pool(name="psC", bufs=2, space="PSUM") as psC,  # proj/y
        ):
            # ---------------- input DMA ----------------
            # weights first (boot chains + warm-up), then x seq-block 0 in
            # per-dc chunks (fine-grained boot pacing), then rope tables for
            # block 0, then the x remainder per dc, then the rest.
            wq_sb = persist.tile([128, N_DC, DQ], bf16, tag="wq")
            nc.sync.dma_start(
                out=wq_sb[:], in_=wq.rearrange("(c p) n -> p c n", p=128)
            )
            wk_sb = persist.tile([128, N_DC, DQ], bf16, tag="wk")
            nc.gpsimd.dma_start(
                out=wk_sb[:], in_=wk.rearrange("(c p) n -> p c n", p=128)
            )
            wv_sb = persist.tile([128, N_DC, DQ], bf16, tag="wv")
            nc.sync.dma_start(
                out=wv_sb[:], in_=wv.rearrange("(c p) n -> p c n", p=128)
            )
            xt_sb = [
                persist.tile([128, S], bf16, tag=f"xt{dc}", name=f"xt{dc}")
                for dc in range(N_DC)
            ]
            for dc in range(N_DC):
                eng = nc.sync if dc % 2 == 0 else nc.gpsimd
                eng.dma_start(
                    out=xt_sb[dc][:, 0:512],
                    in_=xT[128 * dc : 128 * (dc + 1), 0:512],
                )
            cos_sb = persist.tile([128, S], f32, tag="cos")
            sin_sb = persist.tile([128, S], f32, tag="sin")
            nc.sync.dma_start(out=cos_sb[:, 0:512], in_=cosT[:, 0:512])
            nc.gpsimd.dma_start(out=sin_sb[:, 0:512], in_=sinT[:, 0:512])
            mask_sb = persist.tile([128, 256], bf16, tag="mask")
            nc.gpsimd.dma_start(out=mask_sb[:], in_=masks[:])
            for dc in range(N_DC):
                eng = nc.sync if dc % 2 == 0 else nc.gpsimd
                eng.dma_start(
                    out=xt_sb[dc][:, 512:S],
                    in_=xT[128 * dc : 128 * (dc + 1), 512:S],
                )
            nc.sync.dma_start(out=cos_sb[:, 512:S], in_=cosT[:, 512:S])
            nc.gpsimd.dma_start(out=sin_sb[:, 512:S], in_=sinT[:, 512:S])
            wo_sb = persist.tile([128, 2, D], bf16, tag="wo")
            nc.sync.dma_start(
                out=wo_sb[:], in_=wo.rearrange("(c p) n -> p c n", p=128)
            )

            # PE warm-up on a memset tile: no DMA dependency, so the HAM
            # clock ramp (needs ~3.4us of sustained activity for 2.4GHz)
            # starts as soon as the preamble ends, while inputs stream.
            junk = persist.tile([128, 256], bf16, tag="junk")
            nc.vector.memset(junk[:], 0.03125)
            warm_tiles = [
                psA.tile([128, 1024], f32, tag="score", name=f"warm{i}")
                for i in range(2)
            ]
            for wi in range(24):
                nc.tensor.matmul(
                    warm_tiles[wi % 2][:, 0:256],
                    junk[:, 0:128],
                    junk[:],
                    start=True,
                    stop=True,
                )

            # persistent intermediates
            qT_sb = persist.tile([128, 2, S], bf16, tag="qT")  # [64h..., cc, s]
            kT_sb = persist.tile([128, 2, S], bf16, tag="kT")
            v_sb = persist.tile([128, N_KB, H_PER_CORE, HD + 1], bf16, tag="v")
            nc.vector.memset(v_sb[:, :, :, HD : HD + 1], 1.0)
            ctxT_sb = persist.tile([128, 2, S], bf16, tag="ctxT")  # unnormalized
            # denominators staged on one partition (engine writes must start
            # at partition 0/32/64/96); chunk qb*4+hh holds head hh, block qb
            stage_sb = persist.tile([1, H_PER_CORE * S], f32, tag="stage")
            recip_dram = dram_pool.tile([N_SB, H_PER_CORE, 512], bf16, tag="rdram")

            # ---------------- helpers ----------------
            def rope(src_ps, dst_sb, cc, sb):
                """dst = src*cos + rotate_half(src)*sin, fp32 in, bf16 out.

                The rotate-half partition shift is done by small SBUF->SBUF
                DMAs (a [32,512] DVE op costs as much as a [128,512] one, so
                quarter-sized DVE ops waste 3/4 of the lanes; DMA engines are
                otherwise idle).
                """
                ss = slice(512 * sb, 512 * (sb + 1))
                t1 = rope_pool.tile([128, 512], bf16, tag="ropeA", name="t1")
                nc.vector.tensor_mul(t1[:], src_ps, cos_sb[:, ss])
                # sin table is pre-shifted on the host (sinx[p] =
                # sin_signed[partner(p)]) so this product is computed at the
                # SOURCE rows and only then moved to the partner rows by DMA
                t2p = rope_pool.tile([128, 512], bf16, tag="ropeQ", name="t2p")
                nc.vector.tensor_mul(t2p[:], src_ps, sin_sb[:, ss])
                rot = rope_pool.tile([128, 512], bf16, tag="ropeB", name="rot")
                for quarter in range(4):
                    o = 32 * quarter
                    src_o = o + 32 if quarter % 2 == 0 else o - 32
                    nc.gpsimd.dma_start(
                        out=rot[o : o + 32, :], in_=t2p[src_o : src_o + 32, :]
                    )
                nc.vector.tensor_add(dst_sb[:, cc, ss], t1[:], rot[:])

            def proj_q_quantum(cc, sb):
                ss = slice(512 * sb, 512 * (sb + 1))
                q_ps = psC.tile([128, 512], f32, tag="proj", name="q_ps")
                for dc in range(N_DC):
                    nc.tensor.matmul(
                        q_ps[:],
                        wq_sb[:, dc, 128 * cc : 128 * (cc + 1)],
                        xt_sb[dc][:, ss],
                        start=(dc == 0),
                        stop=(dc == N_DC - 1),
                    )
                rope(q_ps[:], qT_sb, cc, sb)

            def proj_k_quantum(cc, sb):
                ss = slice(512 * sb, 512 * (sb + 1))
                k_ps = psC.tile([128, 512], f32, tag="proj", name="k_ps")
                for dc in range(N_DC):
                    nc.tensor.matmul(
                        k_ps[:],
                        wk_sb[:, dc, 128 * cc : 128 * (cc + 1)],
                        xt_sb[dc][:, ss],
                        start=(dc == 0),
                        stop=(dc == N_DC - 1),
                    )
                rope(k_ps[:], kT_sb, cc, sb)

            def proj_v_quantum(sc):
                # PSUM accumulation groups are per-bank, so each sc chain
                # needs its own tile even though it only fills half of it
                v_ps = psC.tile([128, 512], f32, tag="proj", name="v_ps")[
                    :, 0:256
                ]
                for dc in range(N_DC):
                    nc.tensor.matmul(
                        v_ps,
                        xt_sb[dc][:, 128 * sc : 128 * (sc + 1)],
                        wv_sb[:, dc, :],
                        start=(dc == 0),
                        stop=(dc == N_DC - 1),
                    )
                nc.vector.tensor_copy(
                    v_sb[:, sc, :, 0:HD],
                    v_ps.rearrange("p (h d) -> p h d", h=H_PER_CORE),
                )

            def out_proj_quantum(qb, oc, copy_eng=None):
                qs = slice(512 * qb, 512 * (qb + 1))
                y_ps = psC.tile([128, 512], f32, tag="proj", name="y_ps")
                for cc in range(2):
                    nc.tensor.matmul(
                        y_ps[:],
                        wo_sb[:, cc, 128 * oc : 128 * (oc + 1)],
                        ctxT_sb[:, cc, qs],
                        start=(cc == 0),
                        stop=(cc == 1),
                    )
                y_sb = yout_pool.tile([128, 512], bf16, tag="y", name="y_sb")
                nc.vector.tensor_copy(y_sb[:], y_ps[:])
                nc.sync.dma_start(
                    out=yT[128 * oc : 128 * (oc + 1), qs], in_=y_sb[:]
                )

            def attention(cc, qb, fills=()):
                """Causal attention for head pair cc, query block qb.

                Per k-block: two score matmuls (head h in PE row-group h) into
                one [128,1024] PSUM tile, one exp over both heads, triangle
                mask on the diagonal 128 columns, then (one k-block delayed)
                the two AV matmuls accumulating ctx+denominator via the ones
                column. Diagonal k-blocks only touch the valid q >= k range.

                `fills` are independent PE-work quanta, distributed across
                k-block slots to cover the exp dependency chain.
                """
                import collections

                fills = collections.deque(fills)
                nkb = 4 * qb + 4
                ctx_ps = [
                    psB.tile([HD + 1, 512], f32, tag="ctx", name=f"ctx{h}")
                    for h in range(2)
                ]
                pending = None  # (kb, o, a_t) whose AV matmuls haven't run

                def emit_av(kb, o, a_t, stop):
                    for h in range(2):
                        nc.tensor.matmul(
                            ctx_ps[h][:, o:512],
                            v_sb[:, kb, 2 * cc + h, :],
                            a_t[:, 512 * h + o : 512 * (h + 1)],
                            start=(kb == 0),
                            stop=stop,
                            skip_group_check=True,
                        )

                for kb in range(nkb):
                    j = kb - 4 * qb  # >= 0 on diagonal blocks
                    o = 128 * j if j > 0 else 0
                    s_ps = psA.tile([128, 1024], f32, tag="score", name="s_ps")
                    for h in range(2):
                        hp = slice(64 * h, 64 * (h + 1))
                        nc.tensor.matmul(
                            s_ps[:, 512 * h + o : 512 * (h + 1)],
                            kT_sb[hp, cc, 128 * kb : 128 * (kb + 1)],
                            qT_sb[hp, cc, 512 * qb + o : 512 * (qb + 1)],
                            start=True,
                            stop=True,
                        )
                    a_t = attn_pool.tile([128, 1024], bf16, tag="attnT", name="a_t")
                    s3 = s_ps[:].rearrange("p (h q) -> p h q", h=2)
                    a3 = a_t[:].rearrange("p (h q) -> p h q", h=2)
                    nc.scalar.activation(
                        a3[:, :, o:512],
                        s3[:, :, o:512],
                        mybir.ActivationFunctionType.Exp,
                        scale=float(1.0 / np.sqrt(HD)),
                    )
                    if j >= 0:
                        nc.vector.tensor_mul(
                            a3[:, :, o : o + 128],
                            a3[:, :, o : o + 128],
                            mask_sb[:].rearrange("p (c q) -> p c q", c=2),
                        )
                    if pending is not None:
                        emit_av(*pending, stop=False)
                    pending = (kb, o, a_t)
                    # distribute remaining fills over remaining slots
                    nf = (len(fills) + nkb - 1 - kb) // (nkb - kb)
                    for _ in range(nf):
                        fills.popleft()()
                emit_av(*pending, stop=True)
                # stage denominators first (the normalization chain hangs off
                # them), then remaining fills, then the bulk ctx copies
                r0 = qb * H_PER_CORE + 2 * cc
                nc.vector.tensor_copy(
                    stage_sb[0:1, 512 * r0 : 512 * (r0 + 1)],
                    ctx_ps[0][HD : HD + 1, :],
                )
                nc.scalar.copy(
                    stage_sb[0:1, 512 * (r0 + 1) : 512 * (r0 + 2)],
                    ctx_ps[1][HD : HD + 1, :],
                )
                while fills:
                    fills.popleft()()
                for h in range(2):
                    nc.vector.tensor_copy(
                        ctxT_sb[64 * h : 64 * (h + 1), cc, 512 * qb : 512 * (qb + 1)],
                        ctx_ps[h][0:HD, :],
                    )

            def normalize(cc, qb):
                """Reciprocal + broadcast + scale for head pair cc, block qb.

                All DMAs keep >=256B contiguous chunks: a descriptor-per-
                element scatter (e.g. a [128,8]-layout reciprocal store)
                measures ~15us on the DMA engine and stalls the in-order
                ring, cascading head-of-line blocking across every queue.
                """
                # repartition [1, 1024] -> [8, 128] so reciprocal is cheap
                # (reciprocal cost scales with free size only)
                base = (qb * H_PER_CORE + 2 * cc) * 512
                den_q = small_pool.tile([8, 128], f32, tag="den_q", name="den_q")
                nc.sync.dma_start(
                    out=den_q[:], in_=stage_sb[0:1, base : base + 1024]
                )
                rec_q = small_pool.tile([8, 128], bf16, tag="rec_q", name="rec_q")
                with nc.allow_low_precision(
                    reason="bf16 softmax denom matches bf16 attn weights"
                ):
                    nc.vector.reciprocal(rec_q[:], den_q[:])
                if cc == 1 and qb == N_SB - 1:
                    # keep the PE's HAM clock warm through the tail
                    # normalization chain: scratch matmuls gated on the
                    # chain's own data so the scheduler cannot hoist them
                    warm = psA.tile([128, 1024], f32, tag="score", name="warm")
                    for wi in range(8):
                        nc.tensor.matmul(
                            warm[:, 0:512],
                            rec_q[:],
                            xt_sb[wi][0:8, 0:512],
                            start=True,
                            stop=True,
                        )
                nc.sync.dma_start(
                    out=recip_dram[qb, 2 * cc : 2 * cc + 2, :], in_=rec_q[:]
                )
                qs = slice(512 * qb, 512 * (qb + 1))
                bc_sb = small_pool.tile([128, 512], bf16, tag="bcast", name="bc_sb")
                for h in range(2):
                    row = recip_dram[qb, 2 * cc + h, :]
                    bcast = bass.AP(
                        tensor=row.tensor,
                        offset=row.offset,
                        ap=[[0, 64]] + list(row.ap)[-1:],
                    )
                    nc.sync.dma_start(
                        out=bc_sb[64 * h : 64 * (h + 1), :], in_=bcast
                    )
                nc.vector.tensor_mul(
                    ctxT_sb[:, cc, qs], ctxT_sb[:, cc, qs], bc_sb[:]
                )

            # ---------------- boot: projections paced by the x DMAs -------
            # 8 parallel PSUM chains (q/k for both head pairs in the two psA
            # score tiles, 4 V blocks in the two psC tiles) so each arriving
            # x chunk feeds ~1.3us of PE work.
            qk_ps = [
                psA.tile([128, 1024], f32, tag="score", name=f"qk{cc}")
                for cc in range(2)
            ]
            v_boot = [
                psC.tile([128, 512], f32, tag="proj", name=f"vb{i}")
                for i in range(2)
            ]
            for dc in range(N_DC):
                st, sp = dc == 0, dc == N_DC - 1
                for cc in range(2):
                    nc.tensor.matmul(
                        qk_ps[cc][:, 0:512],
                        wq_sb[:, dc, 128 * cc : 128 * (cc + 1)],
                        xt_sb[dc][:, 0:512],
                        start=st,
                        stop=sp,
                    )
                    nc.tensor.matmul(
                        qk_ps[cc][:, 512:1024],
                        wk_sb[:, dc, 128 * cc : 128 * (cc + 1)],
                        xt_sb[dc][:, 0:512],
                        start=st,
                        stop=sp,
                    )
                # only 2 spare banks -> first 2 of the 4 V chains ride along
                for sc in range(2):
                    nc.tensor.matmul(
                        v_boot[sc][:, 0:256],
                        xt_sb[dc][:, 128 * sc : 128 * (sc + 1)],
                        wv_sb[:, dc, :],
                        start=st,
                        stop=sp,
                    )
            for cc in range(2):
                rope(qk_ps[cc][:, 0:512], qT_sb, cc, 0)
                rope(qk_ps[cc][:, 512:1024], kT_sb, cc, 0)
            for sc in range(2):
                nc.vector.tensor_copy(
                    v_sb[:, sc, :, 0:HD],
                    v_boot[sc][:, 0:256].rearrange(
                        "p (h d) -> p h d", h=H_PER_CORE
                    ),
                )
            for sc in range(2, 4):
                proj_v_quantum(sc)

            # ---------------- main pipeline ----------------
            # baseline-style bulk emission order: the ACT-paced attention
            # inner loop leaves the PE ~15-25% idle, which keeps the HAM
            # power governor's activity estimate under its clamp threshold.
            # A densely filler-packed schedule measures ~40% SLOWER: the
            # governor duty-cycles the PE to half rate (k=4/n=8 windows,
            # matmuls stretch ~1.6x) for half the kernel.
            for sb in range(N_SB):
                if sb > 0:
                    proj_q_quantum(1, sb)
                    proj_k_quantum(1, sb)
                    for sc in range(4 * sb, 4 * sb + 4):
                        proj_v_quantum(sc)
                attention(0, sb)
                normalize(0, sb)  # chain covered by attention(1, sb) PE work
                if sb == N_SB - 1:
                    attention(
                        1,
                        sb,
                        fills=[
                            (lambda oc=oc: out_proj_quantum(2, oc))
                            for oc in range(4, N_DC)
                        ],
                    )
                else:
                    attention(1, sb)
                normalize(1, sb)
                if sb < N_SB - 1:
                    # emit the next block's first projection before this
                    # block's out-projection so the reciprocal chain is
                    # covered by PE work and the PE never idles
                    proj_q_quantum(0, sb + 1)
                    proj_k_quantum(0, sb + 1)
                if sb < 2:
                    for oc in range(N_DC):
                        out_proj_quantum(sb, oc)
                elif sb == 2:
                    # hold back half of qb=2's out-projection; it is emitted
                    # as filler inside attention(1, 3) to cover the final
                    # normalization chain
                    for oc in range(0, 4):
                        out_proj_quantum(2, oc)
                else:
                    for oc in range(N_DC):
                        out_proj_quantum(3, oc)

    nc.compile()
    return nc


def _rope_tables():
    inv_freq = (
        1.0 / (THETA ** (np.arange(0, HD, 2, dtype=np.float32) / HD))
    ).astype(np.float32)
    pos = np.arange(S, dtype=np.float32)
    ang = pos[:, None] * inv_freq[None, :]  # [S, 32]
    cos_half = np.cos(ang).astype(np.float32).T  # [32, S]
    sin_half = np.sin(ang).astype(np.float32).T
    # per-head 64 rows: cos rows duplicated. The sin table is PRE-SHIFTED:
    # row p holds sin_signed[partner(p)] (partner = rotate-half swap), so the
    # kernel multiplies at the source rows and a plain partition-shift DMA
    # finishes rotate-half: sinx per head = (+sin | -sin).
    cos64 = np.concatenate([cos_half, cos_half], axis=0)
    sinx64 = np.concatenate([sin_half, -sin_half], axis=0)
    cosT = np.concatenate([cos64, cos64], axis=0)  # [128, S] two heads
    sinT = np.concatenate([sinx64, sinx64], axis=0)
    return np.ascontiguousarray(cosT), np.ascontiguousarray(sinT)


def _masks():
    k = np.arange(128)[:, None]
    q = np.arange(128)[None, :]
    tri = (k <= q).astype(ml_dtypes.bfloat16)
    m = np.empty((128, 256), dtype=ml_dtypes.bfloat16)
    m[:, 0:128] = tri
    m[:, 128:256] = tri
    return m


def kernel(x, W_q, W_k, W_v, W_o):
    global _CACHED
    from concourse.bass_utils import run_bass_kernel_spmd

    if _CACHED is None:
        _CACHED = _build_kernel()
    nc = _CACHED

    bf = ml_dtypes.bfloat16
    cosT, sinT = _rope_tables()
    masks = _masks()
    x = np.asarray(x)
    W_q, W_k, W_v, W_o = (np.asarray(w) for w in (W_q, W_k, W_v, W_o))
    xT = [np.ascontiguousarray(x[b].T).astype(bf) for b in range(B)]

    in_maps = []
    for c in range(N_CORES):
        b, g = divmod(c, 4)
        cols = slice(DQ * g, DQ * (g + 1))
        in_maps.append(
            {
                "xT": xT[b],
                "wq": np.ascontiguousarray(W_q[:, cols]).astype(bf),
                "wk": np.ascontiguousarray(W_k[:, cols]).astype(bf),
                "wv": np.ascontiguousarray(W_v[:, cols]).astype(bf),
                "wo": np.ascontiguousarray(W_o[cols, :]).astype(bf),
                "cosT": cosT,
                "sinT": sinT,
                "masks": masks,
            }
        )

    res = run_bass_kernel_spmd(nc, in_maps, core_ids=list(range(N_CORES)))
    kernel.last_results = res

    y = np.empty((B, S, D), dtype=np.float32)
    for b in range(B):
        acc = res.results[4 * b]["yT"].astype(np.float32)
        for g in range(1, 4):
            acc += res.results[4 * b + g]["yT"].astype(np.float32)
        y[b] = acc.T
    return y


# revision 27
# speedup vs baseline: 1.8156x; 1.1248x over previous
"""Multi-head attention (RoPE, causal) Trainium2 kernel, SPMD over 8 NeuronCores.

Problem: x[2,2048,1024] @ {W_q,W_k,W_v}[1024,1024] -> 16-head causal attention
with RoPE -> @ W_o[1024,1024].

Sharding (batch x heads): core c handles batch b=c//4 and head group g=c%4
(4 heads = 256 of the 1024 qkv dims). Each core computes its heads' QKV
projections, RoPE, causal attention, and a partial out-projection
(ctx_g @ W_o[256g:256g+256, :]). The host sums the 4 partials per batch
(unshard of a partial-sum sharding) and transposes back.

On-device layout is fully transposed ([feature, seq]) so no transposes are
needed anywhere: scores are computed as scoresT[k,q] = K^T.T @ Q^T, the
softmax denominator falls out of the AV matmul via a ones-column appended to
V, and the out-projection consumes ctxT directly.

v2 schedule, all bf16 (fp8 was measured to blow the 2e-2 error budget):
- Causal diagonal restriction: for the 4 diagonal key-blocks of each query
  block, scores/exp/AV only cover the valid q >= k column range (the first
  128 columns of the range get a [128,128] triangle mask); saves PE, ACT and
  DVE work and shrinks the mask input to 64KB.
- Fine-grained software pipeline: attention k-block steps are interleaved
  with "filler" quanta (QK projection halves, V projections, out-projection
  chunks) so the in-order PE queue always has independent work while the
  ACT engine exponentiates. The boot phase runs 8 parallel PSUM accumulation
  chains (q/k for both head pairs + 4 V blocks) paced by the x column-block
  DMA arrivals, with clock-warm matmuls on a memset tile before that.
- Softmax denominators: engine copies stage the PSUM ones-row to SBUF, one
  DMA repartitions both heads to [128,8] (reciprocal cost scales with free
  size only -> ~free), one DMA bounces to DRAM, one stride-0 DMA broadcasts
  across partitions.
"""

import numpy as np
import ml_dtypes

B = 2
S = 2048
D = 1024
H = 16
HD = 64
N_CORES = 8
H_PER_CORE = 4
DQ = H_PER_CORE * HD  # 256 qkv dims per core
N_DC = D // 128  # 8 contraction chunks
N_SB = S // 512  # 4 seq blocks of 512
N_KB = S // 128  # 16 key blocks of 128
THETA = 10000.0

_CACHED = None


def _build_kernel():
    import concourse.bass as bass
    import concourse.mybir as mybir
    import concourse.tile as tile
    from concourse import bacc

    f32 = mybir.dt.float32
    bf16 = mybir.dt.bfloat16

    nc = bacc.Bacc(None, target_bir_lowering=False, num_devices=N_CORES)

    xT = nc.dram_tensor("xT", [D, S], bf16, kind="ExternalInput")
    wq = nc.dram_tensor("wq", [D, DQ], bf16, kind="ExternalInput")
    wk = nc.dram_tensor("wk", [D, DQ], bf16, kind="ExternalInput")
    wv = nc.dram_tensor("wv", [D, DQ], bf16, kind="ExternalInput")
    wo = nc.dram_tensor("wo", [DQ, D], bf16, kind="ExternalInput")
    cosT = nc.dram_tensor("cosT", [128, S], f32, kind="ExternalInput")
    sinT = nc.dram_tensor("sinT", [128, S], f32, kind="ExternalInput")
    # masks[k, 128*c + q] = 1.0 if k <= q else 0 (c=0,1 identical copies)
    masks = nc.dram_tensor("masks", [128, 256], bf16, kind="ExternalInput")
    yT = nc.dram_tensor("yT", [D, S], bf16, kind="ExternalOutput")

    with tile.TileContext(nc) as tc:
        with (
            tc.tile_pool(name="persist", bufs=1) as persist,
            tc.tile_pool(name="attn", bufs=8) as attn_pool,
            tc.tile_pool(name="rope", bufs=4) as rope_pool,
            tc.tile_pool(name="small", bufs=4) as small_pool,
            tc.tile_pool(name="yout", bufs=3) as yout_pool,
            tc.tile_pool(name="dram", bufs=1, space="DRAM") as dram_pool,
            tc.tile_pool(name="psA", bufs=2, space="PSUM") as psA,  # scores 2-bank
            tc.tile_pool(name="psB", bufs=2, space="PSUM") as psB,  # ctx accum
            tc.tile_pool(name="psC", bufs=2, space="PSUM") as psC,  # proj/y
        ):
            # ---------------- input DMA ----------------
            # weights first (boot chains + warm-up), then x seq-block 0 in
            # per-dc chunks (fine-grained boot pacing), then rope tables for
            # block 0, then the x remainder per dc, then the rest.
            wq_sb = persist.tile([128, N_DC, DQ], bf16, tag="wq")
            nc.sync.dma_start(
                out=wq_sb[:], in_=wq.rearrange("(c p) n -> p c n", p=128)
            )
            wk_sb = persist.tile([128, N_DC, DQ], bf16, tag="wk")
            nc.gpsimd.dma_start(
                out=wk_sb[:], in_=wk.rearrange("(c p) n -> p c n", p=128)
            )
            wv_sb = persist.tile([128, N_DC, DQ], bf16, tag="wv")
            nc.sync.dma_start(
                out=wv_sb[:], in_=wv.rearrange("(c p) n -> p c n", p=128)
            )
            xt_sb = [
                persist.tile([128, S], bf16, tag=f"xt{dc}", name=f"xt{dc}")
                for dc in range(N_DC)
            ]
            for dc in range(N_DC):
                eng = nc.sync if dc % 2 == 0 else nc.gpsimd
                eng.dma_start(
                    out=xt_sb[dc][:, 0:512],
                    in_=xT[128 * dc : 128 * (dc + 1), 0:512],
                )
            cos_sb = persist.tile([128, S], f32, tag="cos")
            sin_sb = persist.tile([128, S], f32, tag="sin")
            nc.sync.dma_start(out=cos_sb[:, 0:512], in_=cosT[:, 0:512])
            nc.gpsimd.dma_start(out=sin_sb[:, 0:512], in_=sinT[:, 0:512])
            mask_sb = persist.tile([128, 256], bf16, tag="mask")
            nc.gpsimd.dma_start(out=mask_sb[:], in_=masks[:])
            for dc in range(N_DC):
                eng = nc.sync if dc % 2 == 0 else nc.gpsimd
                eng.dma_start(
                    out=xt_sb[dc][:, 512:S],
                    in_=xT[128 * dc : 128 * (dc + 1), 512:S],
                )
            nc.sync.dma_start(out=cos_sb[:, 512:S], in_=cosT[:, 512:S])
            nc.gpsimd.dma_start(out=sin_sb[:, 512:S], in_=sinT[:, 512:S])
            wo_sb = persist.tile([128, 2, D], bf16, tag="wo")
            nc.sync.dma_start(
                out=wo_sb[:], in_=wo.rearrange("(c p) n -> p c n", p=128)
            )

            # PE warm-up on a memset tile: no DMA dependency, so the HAM
            # clock ramp (needs ~3.4us of sustained activity for 2.4GHz)
            # starts as soon as the preamble ends, while inputs stream.
            junk = persist.tile([128, 256], bf16, tag="junk")
            nc.vector.memset(junk[:], 0.03125)
            warm_tiles = [
                psA.tile([128, 1024], f32, tag="score", name=f"warm{i}")
                for i in range(2)
            ]
            for wi in range(24):
                nc.tensor.matmul(
                    warm_tiles[wi % 2][:, 0:256],
                    junk[:, 0:128],
                    junk[:],
                    start=True,
                    stop=True,
                )

            # persistent intermediates
            qT_sb = persist.tile([128, 2, S], bf16, tag="qT")  # [64h..., cc, s]
            kT_sb = persist.tile([128, 2, S], bf16, tag="kT")
            v_sb = persist.tile([128, N_KB, H_PER_CORE, HD + 1], bf16, tag="v")
            nc.vector.memset(v_sb[:, :, :, HD : HD + 1], 1.0)
            ctxT_sb = persist.tile([128, 2, S], bf16, tag="ctxT")  # unnormalized
            # denominators staged on one partition (engine writes must start
            # at partition 0/32/64/96); chunk qb*4+hh holds head hh, block qb
            stage_sb = persist.tile([1, H_PER_CORE * S], f32, tag="stage")
            recip_dram = dram_pool.tile([N_SB, H_PER_CORE, 512], bf16, tag="rdram")

            # ---------------- helpers ----------------
            def rope(src_ps, dst_sb, cc, sb, shift_eng=None):
                """dst = src*cos + rotate_half(src)*sin, fp32 in, bf16 out.

                The rotate-half partition shift is done by small SBUF->SBUF
                DMAs (a [32,512] DVE op costs as much as a [128,512] one, so
                quarter-sized DVE ops waste 3/4 of the lanes; DMA engines are
                otherwise idle).
                """
                ss = slice(512 * sb, 512 * (sb + 1))
                t1 = rope_pool.tile([128, 512], bf16, tag="ropeA", name="t1")
                nc.vector.tensor_mul(t1[:], src_ps, cos_sb[:, ss])
                # sin table is pre-shifted on the host (sinx[p] =
                # sin_signed[partner(p)]) so this product is computed at the
                # SOURCE rows and only then moved to the partner rows by DMA
                t2p = rope_pool.tile([128, 512], bf16, tag="ropeQ", name="t2p")
                nc.vector.tensor_mul(t2p[:], src_ps, sin_sb[:, ss])
                rot = rope_pool.tile([128, 512], bf16, tag="ropeB", name="rot")
                for quarter in range(4):
                    o = 32 * quarter
                    src_o = o + 32 if quarter % 2 == 0 else o - 32
                    (shift_eng or nc.gpsimd).dma_start(
                        out=rot[o : o + 32, :], in_=t2p[src_o : src_o + 32, :]
                    )
                nc.vector.tensor_add(dst_sb[:, cc, ss], t1[:], rot[:])

            def proj_q_quantum(cc, sb):
                ss = slice(512 * sb, 512 * (sb + 1))
                q_ps = psC.tile([128, 512], f32, tag="proj", name="q_ps")
                for dc in range(N_DC):
                    nc.tensor.matmul(
                        q_ps[:],
                        wq_sb[:, dc, 128 * cc : 128 * (cc + 1)],
                        xt_sb[dc][:, ss],
                        start=(dc == 0),
                        stop=(dc == N_DC - 1),
                    )
                rope(q_ps[:], qT_sb, cc, sb)

            def proj_k_quantum(cc, sb):
                ss = slice(512 * sb, 512 * (sb + 1))
                k_ps = psC.tile([128, 512], f32, tag="proj", name="k_ps")
                for dc in range(N_DC):
                    nc.tensor.matmul(
                        k_ps[:],
                        wk_sb[:, dc, 128 * cc : 128 * (cc + 1)],
                        xt_sb[dc][:, ss],
                        start=(dc == 0),
                        stop=(dc == N_DC - 1),
                    )
                rope(k_ps[:], kT_sb, cc, sb)

            def proj_v_quantum(sc):
                # PSUM accumulation groups are per-bank, so each sc chain
                # needs its own tile even though it only fills half of it
                v_ps = psC.tile([128, 512], f32, tag="proj", name="v_ps")[
                    :, 0:256
                ]
                for dc in range(N_DC):
                    nc.tensor.matmul(
                        v_ps,
                        xt_sb[dc][:, 128 * sc : 128 * (sc + 1)],
                        wv_sb[:, dc, :],
                        start=(dc == 0),
                        stop=(dc == N_DC - 1),
                    )
                nc.vector.tensor_copy(
                    v_sb[:, sc, :, 0:HD],
                    v_ps.rearrange("p (h d) -> p h d", h=H_PER_CORE),
                )

            def out_proj_quantum(qb, oc, copy_eng=None):
                qs = slice(512 * qb, 512 * (qb + 1))
                y_ps = psC.tile([128, 512], f32, tag="proj", name="y_ps")
                for cc in range(2):
                    nc.tensor.matmul(
                        y_ps[:],
                        wo_sb[:, cc, 128 * oc : 128 * (oc + 1)],
                        ctxT_sb[:, cc, qs],
                        start=(cc == 0),
                        stop=(cc == 1),
                    )
                y_sb = yout_pool.tile([128, 512], bf16, tag="y", name="y_sb")
                nc.vector.tensor_copy(y_sb[:], y_ps[:])
                nc.sync.dma_start(
                    out=yT[128 * oc : 128 * (oc + 1), qs], in_=y_sb[:]
                )

            def attention(cc, qb, fills=()):
                """Causal attention for head pair cc, query block qb.

                Per k-block: two score matmuls (head h in PE row-group h) into
                one [128,1024] PSUM tile, one exp over both heads, triangle
                mask on the diagonal 128 columns, then (one k-block delayed)
                the two AV matmuls accumulating ctx+denominator via the ones
                column. Diagonal k-blocks only touch the valid q >= k range.

                `fills` are independent PE-work quanta, distributed across
                k-block slots to cover the exp dependency chain.
                """
                import collections

                fills = collections.deque(fills)
                nkb = 4 * qb + 4
                ctx_ps = [
                    psB.tile([HD + 1, 512], f32, tag="ctx", name=f"ctx{h}")
                    for h in range(2)
                ]
                pending = None  # (kb, o, a_t) whose AV matmuls haven't run

                def emit_av(kb, o, a_t, stop):
                    for h in range(2):
                        nc.tensor.matmul(
                            ctx_ps[h][:, o:512],
                            v_sb[:, kb, 2 * cc + h, :],
                            a_t[:, 512 * h + o : 512 * (h + 1)],
                            start=(kb == 0),
                            stop=stop,
                            skip_group_check=True,
                        )

                for kb in range(nkb):
                    j = kb - 4 * qb  # >= 0 on diagonal blocks
                    o = 128 * j if j > 0 else 0
                    s_ps = psA.tile([128, 1024], f32, tag="score", name="s_ps")
                    for h in range(2):
                        hp = slice(64 * h, 64 * (h + 1))
                        nc.tensor.matmul(
                            s_ps[:, 512 * h + o : 512 * (h + 1)],
                            kT_sb[hp, cc, 128 * kb : 128 * (kb + 1)],
                            qT_sb[hp, cc, 512 * qb + o : 512 * (qb + 1)],
                            start=True,
                            stop=True,
                        )
                    a_t = attn_pool.tile([128, 1024], bf16, tag="attnT", name="a_t")
                    s3 = s_ps[:].rearrange("p (h q) -> p h q", h=2)
                    a3 = a_t[:].rearrange("p (h q) -> p h q", h=2)
                    nc.scalar.activation(
                        a3[:, :, o:512],
                        s3[:, :, o:512],
                        mybir.ActivationFunctionType.Exp,
                        scale=float(1.0 / np.sqrt(HD)),
                    )
                    if j >= 0:
                        nc.vector.tensor_mul(
                            a3[:, :, o : o + 128],
                            a3[:, :, o : o + 128],
                            mask_sb[:].rearrange("p (c q) -> p c q", c=2),
                        )
                    if pending is not None:
                        emit_av(*pending, stop=False)
                    pending = (kb, o, a_t)
                    # distribute remaining fills over remaining slots
                    nf = (len(fills) + nkb - 1 - kb) // (nkb - kb)
                    for _ in range(nf):
                        fills.popleft()()
                emit_av(*pending, stop=True)
                # stage denominators first (the normalization chain hangs off
                # them), then remaining fills, then the bulk ctx copies
                r0 = qb * H_PER_CORE + 2 * cc
                nc.vector.tensor_copy(
                    stage_sb[0:1, 512 * r0 : 512 * (r0 + 1)],
                    ctx_ps[0][HD : HD + 1, :],
                )
                nc.scalar.copy(
                    stage_sb[0:1, 512 * (r0 + 1) : 512 * (r0 + 2)],
                    ctx_ps[1][HD : HD + 1, :],
                )
                while fills:
                    fills.popleft()()
                for h in range(2):
                    nc.vector.tensor_copy(
                        ctxT_sb[64 * h : 64 * (h + 1), cc, 512 * qb : 512 * (qb + 1)],
                        ctx_ps[h][0:HD, :],
                    )

            def normalize(cc, qb):
                """Reciprocal + broadcast + scale for head pair cc, block qb.

                All DMAs keep >=256B contiguous chunks: a descriptor-per-
                element scatter (e.g. a [128,8]-layout reciprocal store)
                measures ~15us on the DMA engine and stalls the in-order
                ring, cascading head-of-line blocking across every queue.
                """
                # repartition [1, 1024] -> [8, 128] so reciprocal is cheap
                # (reciprocal cost scales with free size only)
                base = (qb * H_PER_CORE + 2 * cc) * 512
                den_q = small_pool.tile([8, 128], f32, tag="den_q", name="den_q")
                nc.sync.dma_start(
                    out=den_q[:], in_=stage_sb[0:1, base : base + 1024]
                )
                rec_q = small_pool.tile([8, 128], bf16, tag="rec_q", name="rec_q")
                with nc.allow_low_precision(
                    reason="bf16 softmax denom matches bf16 attn weights"
                ):
                    nc.vector.reciprocal(rec_q[:], den_q[:])
                if cc == 1 and qb == N_SB - 1:
                    # keep the PE's HAM clock warm through the tail
                    # normalization chain: scratch matmuls gated on the
                    # chain's own data so the scheduler cannot hoist them
                    warm = psA.tile([128, 1024], f32, tag="score", name="warm")
                    for wi in range(8):
                        nc.tensor.matmul(
                            warm[:, 0:512],
                            rec_q[:],
                            xt_sb[wi][0:8, 0:512],
                            start=True,
                            stop=True,
                        )
                nc.sync.dma_start(
                    out=recip_dram[qb, 2 * cc : 2 * cc + 2, :], in_=rec_q[:]
                )
                qs = slice(512 * qb, 512 * (qb + 1))
                bc_sb = small_pool.tile([128, 512], bf16, tag="bcast", name="bc_sb")
                for h in range(2):
                    row = recip_dram[qb, 2 * cc + h, :]
                    bcast = bass.AP(
                        tensor=row.tensor,
                        offset=row.offset,
                        ap=[[0, 64]] + list(row.ap)[-1:],
                    )
                    nc.sync.dma_start(
                        out=bc_sb[64 * h : 64 * (h + 1), :], in_=bcast
                    )
                nc.vector.tensor_mul(
                    ctxT_sb[:, cc, qs], ctxT_sb[:, cc, qs], bc_sb[:]
                )

            # ---------------- boot: projections paced by the x DMAs -------
            # 8 parallel PSUM chains (q/k for both head pairs in the two psA
            # score tiles, 4 V blocks in the two psC tiles) so each arriving
            # x chunk feeds ~1.3us of PE work.
            qk_ps = [
                psA.tile([128, 1024], f32, tag="score", name=f"qk{cc}")
                for cc in range(2)
            ]
            v_boot = [
                psC.tile([128, 512], f32, tag="proj", name=f"vb{i}")
                for i in range(2)
            ]
            for dc in range(N_DC):
                st, sp = dc == 0, dc == N_DC - 1
                for cc in range(2):
                    nc.tensor.matmul(
                        qk_ps[cc][:, 0:512],
                        wq_sb[:, dc, 128 * cc : 128 * (cc + 1)],
                        xt_sb[dc][:, 0:512],
                        start=st,
                        stop=sp,
                    )
                    nc.tensor.matmul(
                        qk_ps[cc][:, 512:1024],
                        wk_sb[:, dc, 128 * cc : 128 * (cc + 1)],
                        xt_sb[dc][:, 0:512],
                        start=st,
                        stop=sp,
                    )
                # only 2 spare banks -> first 2 of the 4 V chains ride along
                for sc in range(2):
                    nc.tensor.matmul(
                        v_boot[sc][:, 0:256],
                        xt_sb[dc][:, 128 * sc : 128 * (sc + 1)],
                        wv_sb[:, dc, :],
                        start=st,
                        stop=sp,
                    )
            for cc in range(2):
                # boot ropes shift on the sync queue: the gpsimd queue is
                # busy issuing the x remainder loads at this point
                rope(qk_ps[cc][:, 0:512], qT_sb, cc, 0, shift_eng=nc.sync)
                rope(qk_ps[cc][:, 512:1024], kT_sb, cc, 0, shift_eng=nc.sync)
            for sc in range(2):
                nc.vector.tensor_copy(
                    v_sb[:, sc, :, 0:HD],
                    v_boot[sc][:, 0:256].rearrange(
                        "p (h d) -> p h d", h=H_PER_CORE
                    ),
                )
            for sc in range(2, 4):
                proj_v_quantum(sc)

            # ---------------- main pipeline ----------------
            # proj/out-proj work is distributed into the attention k-block
            # loops as filler quanta so the in-order PE queue has
            # independent work whenever the exp chain would otherwise
            # stall it. V quanta go first (the diagonal k-blocks of the
            # same attention call consume them).
            for sb in range(N_SB):
                fill0 = []
                fill1 = []
                if sb > 0:
                    fill0 += [
                        (lambda sc=sc: proj_v_quantum(sc))
                        for sc in range(4 * sb, 4 * sb + 4)
                    ]
                    fill0.append(lambda sb=sb: proj_q_quantum(1, sb))
                    fill0.append(lambda sb=sb: proj_k_quantum(1, sb))
                    ops = [
                        (lambda oc=oc, sb=sb: out_proj_quantum(sb - 1, oc))
                        for oc in range(N_DC)
                    ]
                    fill0 += ops[0:2]
                    fill1 += ops[2:N_DC]
                if sb < N_SB - 1:
                    fill1.append(lambda sb=sb: proj_q_quantum(0, sb + 1))
                    fill1.append(lambda sb=sb: proj_k_quantum(0, sb + 1))
                attention(0, sb, fill0)
                normalize(0, sb)  # chain covered by attention(1, sb) PE work
                attention(1, sb, fill1)
                normalize(1, sb)
            for oc in range(N_DC):
                out_proj_quantum(N_SB - 1, oc)

    nc.compile()
    return nc


def _rope_tables():
    inv_freq = (
        1.0 / (THETA ** (np.arange(0, HD, 2, dtype=np.float32) / HD))
    ).astype(np.float32)
    pos = np.arange(S, dtype=np.float32)
    ang = pos[:, None] * inv_freq[None, :]  # [S, 32]
    cos_half = np.cos(ang).astype(np.float32).T  # [32, S]
    sin_half = np.sin(ang).astype(np.float32).T
    # per-head 64 rows: cos rows duplicated. The sin table is PRE-SHIFTED:
    # row p holds sin_signed[partner(p)] (partner = rotate-half swap), so the
    # kernel multiplies at the source rows and a plain partition-shift DMA
    # finishes rotate-half: sinx per head = (+sin | -sin).
    cos64 = np.concatenate([cos_half, cos_half], axis=0)
    sinx64 = np.concatenate([sin_half, -sin_half], axis=0)
    cosT = np.concatenate([cos64, cos64], axis=0)  # [128, S] two heads
    sinT = np.concatenate([sinx64, sinx64], axis=0)
    return np.ascontiguousarray(cosT), np.ascontiguousarray(sinT)


def _masks():
    k = np.arange(128)[:, None]
    q = np.arange(128)[None, :]
    tri = (k <= q).astype(ml_dtypes.bfloat16)
    m = np.empty((128, 256), dtype=ml_dtypes.bfloat16)
    m[:, 0:128] = tri
    m[:, 128:256] = tri
    return m


def kernel(x, W_q, W_k, W_v, W_o):
    global _CACHED
    from concourse.bass_utils import run_bass_kernel_spmd

    if _CACHED is None:
        _CACHED = _build_kernel()
    nc = _CACHED

    bf = ml_dtypes.bfloat16
    cosT, sinT = _rope_tables()
    masks = _masks()
    x = np.asarray(x)
    W_q, W_k, W_v, W_o = (np.asarray(w) for w in (W_q, W_k, W_v, W_o))
    xT = [np.ascontiguousarray(x[b].T).astype(bf) for b in range(B)]

    in_maps = []
    for c in range(N_CORES):
        b, g = divmod(c, 4)
        cols = slice(DQ * g, DQ * (g + 1))
        in_maps.append(
            {
                "xT": xT[b],
                "wq": np.ascontiguousarray(W_q[:, cols]).astype(bf),
                "wk": np.ascontiguousarray(W_k[:, cols]).astype(bf),
                "wv": np.ascontiguousarray(W_v[:, cols]).astype(bf),
                "wo": np.ascontiguousarray(W_o[cols, :]).astype(bf),
                "cosT": cosT,
                "sinT": sinT,
                "masks": masks,
            }
        )

    res = run_bass_kernel_spmd(nc, in_maps, core_ids=list(range(N_CORES)))
    kernel.last_results = res

    y = np.empty((B, S, D), dtype=np.float32)
    for b in range(B):
        acc = res.results[4 * b]["yT"].astype(np.float32)
        for g in range(1, 4):
            acc += res.results[4 * b + g]["yT"].astype(np.float32)
        y[b] = acc.T
    return y
